# revision 31
# baseline (speedup 1.0000x reference)
"""LoRA attention processor on 8 NeuronCores (Trainium2, Bass/Tile).

Reference computation (B=2, S=4096, D=1280, H=8 heads, dh=160, rank-4 LoRA
on K/V):
    q = x @ Wq; k = x @ Wk; v = x @ Wv
    k += (k @ Ak) @ Bk; v += (v @ Av) @ Bv        (LoRA, rank 4)
    attn = softmax(q k^T / sqrt(dh)) v   per head
    out = attn @ Wout + b_out

Sharding: core c handles batch b = c//4 and head pair p = c%4. The LoRA
update is folded into the weights on the host. Each core returns, per head,
the UNNORMALIZED projected partial (exp-weighted sums through Wout) plus
the softmax denominator rows; the host applies the 1/den normalization,
sums the 4 head-pair partials per batch, and adds the bias. Keeping the
normalization off-device removes an ACT/DVE serial chain per output tile
that otherwise rate-limits the kernel's tail.

All matmuls run in fp16 (full PE rate). exp() is computed with a constant
-8 shift so e^smax (~15.5), the unnormalized numerators (~6.3e3), and the
projected partials all fit fp16; the shift cancels in the normalization.
Weight columns per head pair are reordered [h0 dims 0:128 | h1 dims 0:128 |
h0 128:160 | h1 128:160] so the dh=160 tails pack: score tail matmuls are
K=32 row-tiles at tile_position (0,0)/(32,0) (concurrent), PV tail matmuls
are M=34 col-tiles at (0,0)/(0,64) (concurrent), with the softmax-
denominator ones column riding in the aug weights. The [k-pos, q-pos]
transposed score layout lets exp run on ACT straight out of PSUM
([128,1024] wide read spanning both heads' banks).
"""

import numpy as np
from contextlib import ExitStack

import concourse.bass as bass
import concourse.tile as tile
from concourse import bacc, mybir
from concourse.bass_utils import run_bass_kernel_spmd

B, S, D = 2, 4096, 1280
H, DH = 8, 160
HP = 320           # head-pair columns per core (2 heads)
N_CORES = 8
SC = 512           # free-dim chunk (q columns)
NSC = S // SC      # 8
CK = 128           # contraction chunk
NCK = D // CK      # 10
NJ = S // 128      # 32 key blocks
F32 = mybir.dt.float32
F16 = mybir.dt.float16
SHIFT = 8.0        # exp(s - SHIFT): fits e^s and unnormalized sums in fp16

_CACHE = {}


def build():
    nc = bacc.Bacc("TRN2", target_bir_lowering=False, debug=False,
                   num_devices=N_CORES)
    xT = nc.dram_tensor("xT", [D, S], F16, kind="ExternalInput").ap()
    wq = nc.dram_tensor("wq", [D, HP], F16, kind="ExternalInput").ap()
    wk = nc.dram_tensor("wk", [D, HP], F16, kind="ExternalInput").ap()
    wv = nc.dram_tensor("wv", [D, HP], F16, kind="ExternalInput").ap()
    # rows: [h0 dims 0:128 | h1 dims 0:128 | h0 tail@0:32 + h1 tail@64:96]
    wo = nc.dram_tensor("wo", [384, D], F16, kind="ExternalInput").ap()
    out0 = nc.dram_tensor("out0", [S, D], F16, kind="ExternalOutput").ap()
    out1 = nc.dram_tensor("out1", [S, D], F16, kind="ExternalOutput").ap()
    den_d = nc.dram_tensor("den", [2, S], F32, kind="ExternalOutput").ap()

    ExpF = mybir.ActivationFunctionType.Exp

    with tile.TileContext(nc) as tc, ExitStack() as top:
        per = top.enter_context(tc.tile_pool(name="per", bufs=1))
        QA0 = per.tile([128, S], F16, name="QA0", tag="QA0")
        QA1 = per.tile([128, S], F16, name="QA1", tag="QA1")
        QB = per.tile([64, S], F16, name="QB", tag="QB")
        KA0 = per.tile([128, S], F16, name="KA0", tag="KA0")
        KA1 = per.tile([128, S], F16, name="KA1", tag="KA1")
        KB = per.tile([64, S], F16, name="KB", tag="KB")
        VA0 = per.tile([128, NJ, 128], F16, name="VA0", tag="VA0")
        VA1 = per.tile([128, NJ, 128], F16, name="VA1", tag="VA1")
        # V tails + denominator ones: [:, j, 0:32]=h0 dims, 32=ones, 33=zero,
        # 34:66=h1 dims, 66=ones, 67=zero  (aug lhsT slices 0:34 and 34:68)
        VB = per.tile([128, NJ, 68], F16, name="VB", tag="VB")
        # unnormalized attention numerators, transposed [dh, q]
        oTA0 = per.tile([128, S], F16, name="oTA0", tag="oTA0")
        oTA1 = per.tile([128, S], F16, name="oTA1", tag="oTA1")
        # tails: h0 dims at partitions 0:32, h1 at 64:96; rest stays zero
        oTB = per.tile([128, S], F16, name="oTB", tag="oTB")
        denrow0 = per.tile([1, S], F32, name="denrow0", tag="denrow0")
        denrow1 = per.tile([1, S], F32, name="denrow1", tag="denrow1")
        bias_t = per.tile([128, 1], F32, name="bias_t", tag="bias_t")
        WO = [per.tile([128, D], F16, name=f"WO{i}", tag=f"WO{i}")
              for i in range(3)]

        # ---- phase 1: projections QT/KT (transposed [dh,q]) + V natural ----
        with ExitStack() as ph1:
            xp = ph1.enter_context(tc.tile_pool(name="xp", bufs=2))
            wp = ph1.enter_context(tc.tile_pool(name="wp", bufs=1))
            pq = ph1.enter_context(tc.tile_pool(name="pq", bufs=2, space="PSUM"))
            pv = ph1.enter_context(tc.tile_pool(name="pv", bufs=2, space="PSUM"))

            # DMA order: wq first, then sc0's x chunks, then wk/wv, then WO —
            # so the first projection matmuls start as early as possible.
            wts = {}
            for c in range(NCK):
                t = wp.tile([CK, HP], F16, name=f"wq_{c}", tag=f"wq_{c}")
                nc.sync.dma_start(t[:], wq[c * CK:(c + 1) * CK, :])
                wts[("wq", c)] = t
            xts0 = []
            for c in range(NCK):
                xt = xp.tile([CK, SC], F16, tag=f"xt{c}")
                nc.sync.dma_start(xt[:], xT[c * CK:(c + 1) * CK, 0:SC])
                xts0.append(xt)
            for nm, src in (("wk", wk), ("wv", wv)):
                for c in range(NCK):
                    t = wp.tile([CK, HP], F16, name=f"{nm}_{c}", tag=f"{nm}_{c}")
                    nc.sync.dma_start(t[:], src[c * CK:(c + 1) * CK, :])
                    wts[(nm, c)] = t
            for i in range(3):
                nc.sync.dma_start(WO[i][:], wo[i * 128:(i + 1) * 128, :])

            nc.vector.memset(bias_t[:], -SHIFT)
            nc.vector.memset(VB[:, :, 32:33], 1.0)
            nc.vector.memset(VB[:, :, 33:34], 0.0)
            nc.vector.memset(VB[:, :, 66:67], 1.0)
            nc.vector.memset(VB[:, :, 67:68], 0.0)
            nc.gpsimd.memset(oTB[:], 0.0)

            # warm the ACT exp table early, off phase 2's critical path
            warm = per.tile([1, 2], F32, name="warm", tag="warm")
            nc.vector.memset(warm[:], 0.0)
            warm2 = per.tile([1, 2], F32, name="warm2", tag="warm2")
            nc.scalar.activation(warm2[:], warm[:], ExpF)

            qk_dsts = {"wq": (QA0, QA1, QB), "wk": (KA0, KA1, KB)}
            for sc in range(NSC):
                ss = slice(sc * SC, (sc + 1) * SC)
                if sc == 0:
                    xts = xts0
                else:
                    xts = []
                    for c in range(NCK):
                        xt = xp.tile([CK, SC], F16, tag=f"xt{c}")
                        nc.sync.dma_start(xt[:], xT[c * CK:(c + 1) * CK, ss])
                        xts.append(xt)
                for nm in ("wq", "wk"):
                    dA0, dA1, dB = qk_dsts[nm]
                    psA0 = pq.tile([128, SC], F32, tag="pA0")
                    psA1 = pq.tile([128, SC], F32, tag="pA1")
                    psB = pq.tile([64, SC], F32, tag="pB")
                    for c in range(NCK):
                        st, sp_ = (c == 0), (c == NCK - 1)
                        nc.tensor.matmul(psA0[:], wts[(nm, c)][:, 0:128],
                                         xts[c][:], start=st, stop=sp_)
                    for c in range(NCK):
                        st, sp_ = (c == 0), (c == NCK - 1)
                        nc.tensor.matmul(psA1[:], wts[(nm, c)][:, 128:256],
                                         xts[c][:], start=st, stop=sp_)
                    for c in range(NCK):
                        st, sp_ = (c == 0), (c == NCK - 1)
                        nc.tensor.matmul(psB[:], wts[(nm, c)][:, 256:320],
                                         xts[c][:], start=st, stop=sp_)
                    nc.any.tensor_copy(dA0[:, ss], psA0[:])
                    nc.any.tensor_copy(dA1[:, ss], psA1[:])
                    nc.any.tensor_copy(dB[:, ss], psB[:])
                # V natural: psum[s, d] = x[c, s].T @ wv[c, :]
                for st4 in range(4):
                    s0 = sc * 4 + st4
                    psV = pv.tile([128, HP], F32, tag="pV")
                    for c in range(NCK):
                        nc.tensor.matmul(
                            psV[:], xts[c][:, st4 * 128:(st4 + 1) * 128],
                            wts[("wv", c)][:], start=(c == 0), stop=(c == NCK - 1))
                    nc.vector.tensor_copy(VA0[:, s0, :], psV[:, 0:128])
                    nc.vector.tensor_copy(VA1[:, s0, :], psV[:, 128:256])
                    nc.vector.tensor_copy(VB[:, s0, 0:32], psV[:, 256:288])
                    nc.vector.tensor_copy(VB[:, s0, 34:66], psV[:, 288:320])

        # ---- phase 2: attention, both heads together per q-chunk ----
        with ExitStack() as ph2:
            scp = ph2.enter_context(tc.tile_pool(name="scp", bufs=2, space="PSUM"))
            ovp = ph2.enter_context(tc.tile_pool(name="ovp", bufs=1, space="PSUM"))
            obp = ph2.enter_context(tc.tile_pool(name="obp", bufs=2, space="PSUM"))
            ep = ph2.enter_context(tc.tile_pool(name="ep", bufs=3))

            for qc in range(NSC):
                qs = slice(qc * SC, (qc + 1) * SC)
                oA0 = ovp.tile([128, SC], F32, tag="oA0")
                oA1 = ovp.tile([128, SC], F32, tag="oA1")
                # [0:32]=h0 tail dims, 32=h0 den, [64:96]=h1 tail, 96=h1 den
                oB = obp.tile([128, SC], F32, tag="oB")

                sc_tiles = {}

                def emit_scores(j):
                    js = slice(j * 128, (j + 1) * 128)
                    scps = scp.tile([128, 2 * SC], F32, tag="sc")
                    sc_tiles[j] = scps
                    nc.tensor.matmul(scps[:, 0:SC], KA0[:, js], QA0[:, qs],
                                     start=True, stop=False,
                                     skip_group_check=True)
                    nc.tensor.matmul(scps[:, SC:2 * SC], KA1[:, js], QA1[:, qs],
                                     start=True, stop=False,
                                     skip_group_check=True)
                    nc.tensor.matmul(scps[:, 0:SC], KB[0:32, js], QB[0:32, qs],
                                     start=False, stop=True,
                                     tile_position=(0, 0),
                                     skip_group_check=True)
                    nc.tensor.matmul(scps[:, SC:2 * SC], KB[32:64, js],
                                     QB[32:64, qs], start=False, stop=True,
                                     tile_position=(32, 0),
                                     skip_group_check=True)

                emit_scores(0)
                for j in range(NJ):
                    scps = sc_tiles.pop(j)
                    ex = ep.tile([128, 2 * SC], F16, tag="ex")
                    nc.scalar.activation(ex[:], scps[:], ExpF, bias=bias_t[:])
                    if j < NJ - 1:
                        emit_scores(j + 1)
                    st, sp_ = (j == 0), (j == NJ - 1)
                    nc.tensor.matmul(oA0[:], VA0[:, j, :], ex[:, 0:SC],
                                     start=st, stop=sp_, skip_group_check=True)
                    nc.tensor.matmul(oA1[:], VA1[:, j, :], ex[:, SC:2 * SC],
                                     start=st, stop=sp_, skip_group_check=True)
                    nc.tensor.matmul(oB[0:34, :], VB[:, j, 0:34], ex[:, 0:SC],
                                     start=st, stop=sp_, tile_position=(0, 0),
                                     skip_group_check=True)
                    nc.tensor.matmul(oB[64:98, :], VB[:, j, 34:68],
                                     ex[:, SC:2 * SC], start=st, stop=sp_,
                                     tile_position=(0, 64),
                                     skip_group_check=True)

                # denominators out (host applies 1/den), numerators to fp16
                nc.vector.tensor_copy(denrow0[:, qs], oB[32:33, :])
                nc.vector.tensor_copy(denrow1[:, qs], oB[96:97, :])
                nc.vector.tensor_copy(oTA0[:, qs], oA0[:])
                nc.vector.tensor_copy(oTA1[:, qs], oA1[:])
                nc.vector.tensor_copy(oTB[0:32, qs], oB[0:32, :])
                nc.vector.tensor_copy(oTB[64:96, qs], oB[64:96, :])

            nc.sync.dma_start(den_d[0:1, :], denrow0[:])
            nc.sync.dma_start(den_d[1:2, :], denrow1[:])

        # ---- phase 3: per-head output projection (unnormalized partials) ----
        with ExitStack() as ph3:
            fp = ph3.enter_context(tc.tile_pool(name="fp", bufs=3, space="PSUM"))
            fs = ph3.enter_context(tc.tile_pool(name="fs", bufs=2))
            for rq in range(S // 128):
                rs = slice(rq * 128, (rq + 1) * 128)
                osb0 = fs.tile([128, D], F16, tag="osb0")
                osb1 = fs.tile([128, D], F16, tag="osb1")
                for oc, osz in ((0, 512), (512, 512), (1024, 256)):
                    psH0 = fp.tile([128, osz], F32, tag="psH0")
                    psH1 = fp.tile([128, osz], F32, tag="psH1")
                    nc.tensor.matmul(psH0[:], oTA0[:, rs], WO[0][:, oc:oc + osz],
                                     start=True, stop=False,
                                     skip_group_check=True)
                    nc.tensor.matmul(psH1[:], oTA1[:, rs], WO[1][:, oc:oc + osz],
                                     start=True, stop=False,
                                     skip_group_check=True)
                    nc.tensor.matmul(psH0[:], oTB[0:32, rs],
                                     WO[2][0:32, oc:oc + osz],
                                     start=False, stop=True,
                                     tile_position=(0, 0),
                                     skip_group_check=True)
                    nc.tensor.matmul(psH1[:], oTB[64:96, rs],
                                     WO[2][64:96, oc:oc + osz],
                                     start=False, stop=True,
                                     tile_position=(64, 0),
                                     skip_group_check=True)
                    nc.scalar.copy(osb0[:, oc:oc + osz], psH0[:])
                    nc.vector.tensor_copy(osb1[:, oc:oc + osz], psH1[:])
                nc.sync.dma_start(out0[rs, :], osb0[:])
                nc.sync.dma_start(out1[rs, :], osb1[:])

    nc.compile()
    return nc


def kernel(hidden_states, w_q, w_k, w_v, lora_k_a, lora_k_b,
           lora_v_a, lora_v_b, w_out, b_out):
    f64 = np.float64
    wk_eff = (w_k.astype(f64)
              + w_k.astype(f64) @ lora_k_a.astype(f64) @ lora_k_b.astype(f64)
              ).astype(np.float32)
    wv_eff = (w_v.astype(f64)
              + w_v.astype(f64) @ lora_v_a.astype(f64) @ lora_v_b.astype(f64)
              ).astype(np.float32)
    wq_s = (w_q.astype(f64) / np.sqrt(DH)).astype(np.float32)
    w_out = np.asarray(w_out, np.float32)

    def pack_cols(w, h0, h1):
        # [h0 dims 0:128 | h1 dims 0:128 | h0 dims 128:160 | h1 dims 128:160]
        return np.concatenate([
            w[:, h0 * DH:h0 * DH + 128], w[:, h1 * DH:h1 * DH + 128],
            w[:, h0 * DH + 128:(h0 + 1) * DH],
            w[:, h1 * DH + 128:(h1 + 1) * DH]], axis=1)

    xT = [np.ascontiguousarray(np.asarray(hidden_states)[b].T
                               ).astype(np.float16) for b in range(B)]

    in_maps = []
    for c in range(N_CORES):
        b, p = c // 4, c % 4
        h0, h1 = 2 * p, 2 * p + 1
        wo_pack = np.zeros((384, D), np.float32)
        wo_pack[0:128] = w_out[h0 * DH:h0 * DH + 128]
        wo_pack[128:256] = w_out[h1 * DH:h1 * DH + 128]
        wo_pack[256:288] = w_out[h0 * DH + 128:(h0 + 1) * DH]
        wo_pack[320:352] = w_out[h1 * DH + 128:(h1 + 1) * DH]
        in_maps.append({
            "xT": xT[b],
            "wq": np.ascontiguousarray(pack_cols(wq_s, h0, h1)).astype(np.float16),
            "wk": np.ascontiguousarray(pack_cols(wk_eff, h0, h1)).astype(np.float16),
            "wv": np.ascontiguousarray(pack_cols(wv_eff, h0, h1)).astype(np.float16),
            "wo": wo_pack.astype(np.float16),
        })

    global _last_in_maps
    _last_in_maps = in_maps
    if "nc" not in _CACHE:
        _CACHE["nc"] = build()
    res = run_bass_kernel_spmd(_CACHE["nc"], in_maps, list(range(N_CORES)))

    out = np.zeros((B, S, D), np.float32)
    for c in range(N_CORES):
        r = res.results[c]
        rec = 1.0 / r["den"].astype(np.float32)          # [2, S]
        out[c // 4] += (r["out0"].astype(np.float32) * rec[0][:, None]
                        + r["out1"].astype(np.float32) * rec[1][:, None])
    out += np.asarray(b_out, np.float32)
    return out
